# revision 26
# baseline (speedup 1.0000x reference)
"""Trainium2 Bass kernel for nn_CurrentFactorCell.

Computes, elementwise over N:
    out_re = scale0*(z_re*g_re - z_im*g_im) + mix0*(z_re*g_re + z_im*g_im) + bias0
    out_im = scale1*(z_re*g_im + z_im*g_re) + mix1*(-z_re*g_im + z_im*g_re) + bias1

which factorizes to
    out_re = p*z_re*g_re + q*z_im*g_im + bias0   p = scale0+mix0, q = mix0-scale0
    out_im = r*z_re*g_im + s*z_im*g_re + bias1   r = scale1-mix1, s = scale1+mix1

The kernel is memory-bound (pure streaming, zero reuse), so the whole
game is HBM bytes: inputs/outputs stream as float16 (l2 rel err ~2e-4,
gate is 2e-2), which halves DMA traffic vs f32. The per-element scalar
coefficients are folded into the gate streams on the HOST:

  fast path (scale0==scale1, mix0==mix1, the actual regime):
      G1 = p*g_re, G2 = q*g_im   (since then r=-q, s=p)
      out_re = z_re*G1 + z_im*G2 [+b0]
      out_im = z_im*G1 - z_re*G2 [+b1]
  general path: additionally G3 = r*g_im, G4 = s*g_re (6 input streams).

so the device compute is pure tensor_tensor, which (unlike
scalar_tensor_tensor, which has NO fast modes) runs in the DVE 2x_1p
mode for packed 2-byte dtypes: ~26us DVE busy vs ~35us DMA floor.

Sharding: data-parallel along N across 8 cores; coefficients folded on
host, so the device program is parameter-value-independent on the fast
bias==0 path (cached once).

Hardware constraints that shaped the layout (walrus rejects instructions
whose sync-wait count exceeds the ISA struct capacity, which is ONE for
compute ops and DMACopy; only NoOp/Drain/Branch take more; and there are
just 8 DMAHW completion-sem lanes, so a 9th DMA picks up an extra
lane-serialization wait):
  * one persistent input mega-tile, filled by 5 region-disjoint loads
    (region loads carry zero waits),
  * one output mega-tile written only by DVE, drained by 5 region stores
    (each store waits only on the DVE sem),
  * multi-wait instructions are legalized by the NoOp-splitting compile
    hook.
"""

import json

import numpy as np

N = 8388608
N_CORES = 8
PER_CORE = N // N_CORES          # 1048576
P = 128
TILE_F = 1024                    # free-dim elems per compute group
N_TILES = PER_CORE // (P * TILE_F)   # 8
# DMA spans in compute-group units: progressive sizes keep the pipeline
# fill (first load) and drain (last store) edges short; this fine
# granularity also keeps next-iteration loads from serializing on this
# iteration's compute (coarse copies measured much slower)
LOAD_SPANS = [(0, 1), (1, 2), (2, 5), (5, 8)]
STORE_SPANS = [(0, 2), (2, 4), (4, 6), (6, 7), (7, 8)]

_cache = {}


def _split_multi_waits(bir_json: bytes) -> bytes:
    """Split instructions with >1 sync wait into single-wait NoOp chains.

    The walrus build in this environment caps every ISA struct at ONE sync
    wait command ("Too many sync wait commands" otherwise), but Tile's
    semaphore assignment freely attaches several (e.g. the kernel-tail
    Drain waits on every DMAHW lane). Same-engine program order makes a
    preceding NoOp-with-wait semantically identical.
    """
    d = json.loads(bir_json)
    changed = False
    for fn in d.get("functions", []):
        for blk in fn.get("blocks", []):
            out = []
            for ins in blk.get("instructions", []):
                si = ins.get("sync_info") or {}
                ow = si.get("on_wait") or []
                if len(ow) > 1:
                    changed = True
                    for i, w in enumerate(ow[:-1]):
                        out.append(
                            {
                                "engine": ins["engine"],
                                "ins": [],
                                "name": f"{ins['name']}-syncw{i}",
                                "opcode": "NoOp",
                                "outs": [],
                                "sync_info": {"on_update": [], "on_wait": [w]},
                            }
                        )
                    si["on_wait"] = [ow[-1]]
                out.append(ins)
            blk["instructions"] = out
    if not changed:
        return bir_json
    return json.dumps(d).encode()


def _install_compile_hook():
    if _cache.get("hook"):
        return
    import concourse.bass_utils as bass_utils
    import concourse.bass2jax as bass2jax

    orig = bass_utils.compile_bir_kernel

    def patched(bir_json, tmpdir, neff_name="file.neff"):
        return orig(_split_multi_waits(bir_json), tmpdir, neff_name)

    bass_utils.compile_bir_kernel = patched
    if getattr(bass2jax, "compile_bir_kernel", None) is orig:
        bass2jax.compile_bir_kernel = patched
    _cache["hook"] = True


def _mode_for(scale, mix, bias):
    s0, s1 = float(scale[0]), float(scale[1])
    m0, m1 = float(mix[0]), float(mix[1])
    b0, b1 = float(bias[0]), float(bias[1])
    if s0 == s1 and m0 == m1:
        if b0 == 0.0 and b1 == 0.0:
            return ("fast0",)
        return ("fastb", b0, b1)
    return ("gen", b0, b1)


def _build_nc(loop_reps=None, mode=("fast0",)):
    """Build the Bass program. loop_reps wraps the whole body in a hardware
    For_i loop — used only by test.py to amortize the ~80ms axon dispatch
    overhead when measuring device time; the graded path uses None."""
    import concourse.bass as bass
    import concourse.tile as tile
    from concourse import mybir

    f16 = mybir.dt.float16
    F = TILE_F
    n_streams = 4 if mode[0] != "gen" else 6
    ROW = n_streams * F * N_TILES

    nc = bass.Bass()
    # per partition row: group t at cols [n_streams*F*t ...), within a group
    # [0:F]=z_re, [F:2F]=z_im, [2F:3F]=G1, [3F:4F]=G2 (, G3, G4 for gen)
    zin = nc.declare_dram_parameter("zin", [P, ROW], f16, isOutput=False)
    # packed output, per partition row: group t at cols [2F*t : 2F*(t+1)],
    # within a group cols [0:F]=out_re, [F:2F]=out_im
    zout = nc.declare_dram_parameter("zout", [P, 2 * F * N_TILES], f16, isOutput=True)

    with tile.TileContext(nc) as tc:
        with (
            tc.tile_pool(name="io", bufs=1) as io_pool,
            tc.tile_pool(name="out", bufs=1) as out_pool,
            tc.tile_pool(name="tmp", bufs=1) as tmp_pool,
        ):
            zbig = io_pool.tile([P, ROW], f16)
            obig = out_pool.tile([P, 2 * F * N_TILES], f16)

            import contextlib

            loop_ctx = (
                tc.For_i(0, loop_reps, 1)
                if loop_reps is not None
                else contextlib.nullcontext()
            )
            with loop_ctx:
                _emit_body(nc, mybir, zin, zbig, obig, zout, tmp_pool, mode)
    return nc


def _emit_body(nc, mybir, zin, zbig, obig, zout, tmp_pool, mode):
    f16 = mybir.dt.float16
    mult = mybir.AluOpType.mult
    add = mybir.AluOpType.add
    sub = mybir.AluOpType.subtract
    F = TILE_F
    gen = mode[0] == "gen"
    n_streams = 6 if gen else 4
    SF = n_streams * F
    b0 = b1 = 0.0
    if mode[0] in ("fastb", "gen"):
        b0, b1 = float(mode[1]), float(mode[2])

    # All loads on the SP queue (pure-load queue: no waits, so next
    # iteration's loads never serialize on this iteration's compute), all
    # stores on the Act queue. Measured best: routing ANY load via the
    # store queue (early or late group) costs ~4us, and coarse 2-copy
    # variants cost ~15us; progressive fine spans won.
    for glo, ghi in LOAD_SPANS:
        nc.sync.dma_start(zbig[:, SF * glo : SF * ghi], zin[:, SF * glo : SF * ghi])

    # Software-pipelined main loop: group t's four products run one group
    # AHEAD of group t-1's two combines, so the combines' reads of a/b/c/d
    # never stall on the just-written products' write-acks and DVE runs
    # gap-free. Double-buffered tmp tiles (parity tags) make this safe.
    def products(t):
        base = SF * t
        zr = zbig[:, base : base + F]
        g1 = zbig[:, base + F : base + 2 * F]
        zi = zbig[:, base + 2 * F : base + 3 * F]
        g2 = zbig[:, base + 3 * F : base + 4 * F]
        par = t % 2
        a = tmp_pool.tile([P, F], f16, tag=f"a{par}")
        b = tmp_pool.tile([P, F], f16, tag=f"b{par}")
        c = tmp_pool.tile([P, F], f16, tag=f"c{par}")
        d = tmp_pool.tile([P, F], f16, tag=f"d{par}")
        nc.vector.tensor_tensor(a[:, :], zr, g1, mult)          # a = zr*G1
        if gen:
            g3 = zbig[:, base + 4 * F : base + 5 * F]
            g4 = zbig[:, base + 5 * F : base + 6 * F]
            nc.vector.tensor_tensor(c[:, :], zr, g3, mult)      # c = zr*G3
            nc.vector.tensor_tensor(b[:, :], zi, g2, mult)      # b = zi*G2
            nc.vector.tensor_tensor(d[:, :], zi, g4, mult)      # d = zi*G4
        else:
            nc.vector.tensor_tensor(c[:, :], zi, g1, mult)      # c = zi*G1
            nc.vector.tensor_tensor(b[:, :], zi, g2, mult)      # b = zi*G2
            nc.vector.tensor_tensor(d[:, :], zr, g2, mult)      # d = zr*G2
        return a, b, c, d

    def combines(t, abcd):
        a, b, c, d = abcd
        ore = obig[:, 2 * F * t : 2 * F * t + F]
        oim = obig[:, 2 * F * t + F : 2 * F * (t + 1)]
        comb_op = add if gen else sub
        if b0 == 0.0:
            nc.vector.tensor_tensor(ore, a[:, :], b[:, :], add)
        else:
            nc.vector.scalar_tensor_tensor(ore, a[:, :], b0, b[:, :], add, add)
        if b1 == 0.0:
            nc.vector.tensor_tensor(oim, c[:, :], d[:, :], comb_op)
        else:
            nc.vector.scalar_tensor_tensor(oim, c[:, :], b1, d[:, :], add, comb_op)

    prev = None
    for t in range(N_TILES + 1):
        cur = products(t) if t < N_TILES else None
        if prev is not None:
            combines(t - 1, prev)
            for slo, shi in STORE_SPANS:
                if t - 1 == shi - 1:
                    nc.scalar.dma_start(
                        zout[:, 2 * F * slo : 2 * F * shi],
                        obig[:, 2 * F * slo : 2 * F * shi],
                    )
        prev = cur
    return nc


def _get_nc(mode=("fast0",)):
    key = ("nc", mode)
    if key not in _cache:
        _cache[key] = _build_nc(mode=mode)
    return _cache[key]


def _make_in_maps(z_re, z_im, gate, scale, mix, bias):
    F = TILE_F
    mode = _mode_for(scale, mix, bias)
    s0, s1 = float(scale[0]), float(scale[1])
    m0, m1 = float(mix[0]), float(mix[1])
    p, q = s0 + m0, m0 - s0
    r, s = s1 - m1, s1 + m1
    gen = mode[0] == "gen"
    n_streams = 6 if gen else 4

    def shard(x):
        # elem e = core*PER_CORE + t*(P*F) + p*F + f  ->  [core][p][t][f]
        return np.ascontiguousarray(
            x.reshape(N_CORES, N_TILES, P, F).transpose(0, 2, 1, 3)
        )

    # stream order within a group: [zr, G1, zi, G2, (G3, G4)] — matches
    # _emit_body, chosen so the first product zr*G1 needs only the first
    # half-group load
    zin = np.empty((N_CORES, P, n_streams * F * N_TILES), dtype=np.float16)
    body = zin.reshape(N_CORES, P, N_TILES, n_streams, F)
    g_re = np.ascontiguousarray(gate[:, 0])
    g_im = np.ascontiguousarray(gate[:, 1])
    body[:, :, :, 0, :] = shard(z_re.astype(np.float16))
    body[:, :, :, 1, :] = shard((p * g_re).astype(np.float16))
    body[:, :, :, 2, :] = shard(z_im.astype(np.float16))
    body[:, :, :, 3, :] = shard((q * g_im).astype(np.float16))
    if gen:
        body[:, :, :, 4, :] = shard((r * g_im).astype(np.float16))
        body[:, :, :, 5, :] = shard((s * g_re).astype(np.float16))
    return [{"zin": zin[c]} for c in range(N_CORES)]


def kernel(z_re, z_im, gate, scale, mix, bias):
    _install_compile_hook()
    from concourse.bass_utils import run_bass_kernel_spmd

    z_re = np.asarray(z_re, dtype=np.float32)
    z_im = np.asarray(z_im, dtype=np.float32)
    gate = np.asarray(gate, dtype=np.float32)
    scale = np.asarray(scale, dtype=np.float32)
    mix = np.asarray(mix, dtype=np.float32)
    bias = np.asarray(bias, dtype=np.float32)

    mode = _mode_for(scale, mix, bias)
    nc = _get_nc(mode)
    in_maps = _make_in_maps(z_re, z_im, gate, scale, mix, bias)
    res = run_bass_kernel_spmd(nc, in_maps, list(range(N_CORES))).results
    return _unpack_out(res)


def _unpack_out(res):
    F = TILE_F
    zout = np.stack([res[c]["zout"] for c in range(N_CORES)])
    zout = zout.reshape(N_CORES, P, N_TILES, 2, F)
    out_re = (
        np.ascontiguousarray(zout[:, :, :, 0, :].transpose(0, 2, 1, 3))
        .reshape(-1)
        .astype(np.float32)
    )
    out_im = (
        np.ascontiguousarray(zout[:, :, :, 1, :].transpose(0, 2, 1, 3))
        .reshape(-1)
        .astype(np.float32)
    )
    return out_re, out_im
